# revision 11
# baseline (speedup 1.0000x reference)
"""Bass/Trainium2 kernel for nn_BasicLSTM: LayerNorm-LSTM cell, B=16384, dims 1024.

Strategy (data-parallel over 8 cores, batch-sharded, weights replicated):
  Per core: batch 2048, feature-major gate computation.
    - LN(x), LN(h_prev) batch-major (stats via ACT accum_out + DVE reduce),
      normalize split across DVE/ACT, output fp16 to DRAM scratch, then one
      big contiguous DMA-transpose per half-b4-block into actT [feat, batch].
      LN affine is folded into gate weights/bias on the host.
    - Gate matmuls fp16: psum[128, 2048] = [f|i|o|c] slices per (oc, b4);
      contraction K=17x128 where k=16 is a ones-row bias tile (bias folded
      into the matmul). Evacuation: ONE fused sigmoid over [f|i|o] (1536)
      + tanh over c' -- no per-gate ACT bias needed.
    - Elementwise c_t/h_t feature-major (c_prev host-transposed).
    - y matmul fp16 (K=9x128, bias-in-matmul), 4 n-chunks share one
      psum [128, 2048]; Erf directly on psum; gelu finished on DVE.
    - Outputs feature-major [1024, 2048]; host transposes back.
"""
import os
import numpy as np

B, IN, H, OUT = 16384, 1024, 1024, 1024
NCORES = 8
BC = B // NCORES          # 2048 rows per core
EPS = 1e-5
P = 128
KT_G = 17                 # 16 contraction k-tiles + 1 bias tile
KT_Y = 9                  # 8 + 1 bias tile
NB4 = BC // 512           # 4
NOC = 8                   # gate-feature 128-chunks

_CACHE = {}
_last_results = None


def _build_nc():
    import concourse.tile as tile
    from concourse import bacc, mybir
    from concourse._compat import get_trn_type

    f32 = mybir.dt.float32
    f16 = mybir.dt.float16
    AF = mybir.ActivationFunctionType
    ALU = mybir.AluOpType

    nc = bacc.Bacc(get_trn_type() or "TRN2", target_bir_lowering=False)

    x_d = nc.dram_tensor("x", (BC, IN), f32, kind="ExternalInput")
    h_d = nc.dram_tensor("h_prev", (BC, H), f32, kind="ExternalInput")
    cT_d = nc.dram_tensor("c_prevT", (H, BC), f32, kind="ExternalInput")
    wg_d = nc.dram_tensor("Wg", (NOC, KT_G, P, 512), f16, kind="ExternalInput")
    why_d = nc.dram_tensor("WhyT", (KT_Y, P, OUT), f16, kind="ExternalInput")

    ct_o = nc.dram_tensor("c_tT", (H, BC), f32, kind="ExternalOutput")
    ht_o = nc.dram_tensor("h_tT", (H, BC), f32, kind="ExternalOutput")
    yt_o = nc.dram_tensor("y_tT", (OUT, BC), f32, kind="ExternalOutput")

    SQ2I = float(1.0 / np.sqrt(2.0))

    with tile.TileContext(nc) as tc:
        with (
            tc.tile_pool(name="const", bufs=1) as const,
            tc.tile_pool(name="actT", bufs=NB4) as actT_pool,
            tc.tile_pool(name="h16", bufs=NB4) as h16_pool,
            tc.tile_pool(name="wg", bufs=2) as wg_pool,
            tc.tile_pool(name="why", bufs=1) as why_pool,
            tc.tile_pool(name="xh", bufs=3) as xh_pool,
            tc.tile_pool(name="sqp", bufs=1) as sq_pool,
            tc.tile_pool(name="lnst", bufs=6) as lnst_pool,
            tc.tile_pool(name="xn16", bufs=3) as xn16_pool,
            tc.tile_pool(name="gate", bufs=2) as gate_pool,
            tc.tile_pool(name="cpv", bufs=3) as cpv_pool,
            tc.tile_pool(name="outs", bufs=2) as outs_pool,
            tc.tile_pool(name="ps", bufs=2, space="PSUM") as ps_pool,
            tc.tile_pool(name="zdram", bufs=1, space="DRAM") as zdram_pool,
        ):
            # ones-row rhs for the bias matmuls: row0 = 1, rest 0
            ones_rhs = const.tile([P, 512], f16)
            nc.vector.memset(ones_rhs[:], 0.0)
            nc.vector.memset(ones_rhs[0:1, :], 1.0)

            actT = [actT_pool.tile([P, 16, 512], f16, tag="actT", name=f"actT{i}")
                    for i in range(NB4)]
            h16 = [h16_pool.tile([P, 8, 512], f16, tag="h16", name=f"h16_{i}")
                   for i in range(NB4)]

            why = why_pool.tile([P, KT_Y, 1024], f16, tag="why")
            nc.gpsimd.dma_start(why[:], why_d.rearrange("ko p c -> p ko c"))

            zx_dram = zdram_pool.tile([BC, 1024], f16, name="zx_dram")
            zh_dram = zdram_pool.tile([BC, 1024], f16, name="zh_dram")

            # ---- Phase 1: LN, batch-major; fp16 out to DRAM; transpose per half-b4
            def ln_panel(src_d, m, which):
                xp = xh_pool.tile([P, 1024], f32, tag="xh")
                nc.gpsimd.dma_start(xp[:], src_d[m * P:(m + 1) * P, :])
                s1 = lnst_pool.tile([P, 1], f32, tag="s1")
                s2 = lnst_pool.tile([P, 1], f32, tag="s2")
                sq = sq_pool.tile([P, 1024], f32, tag="sq")
                nc.scalar.activation(sq[:], xp[:], AF.Square, accum_out=s2[:])
                nc.vector.reduce_sum(s1[:], xp[:], axis=mybir.AxisListType.X)
                mu = lnst_pool.tile([P, 1], f32, tag="mu")
                nc.vector.tensor_scalar_mul(mu[:], s1[:], 1.0 / 1024.0)
                ve = lnst_pool.tile([P, 1], f32, tag="ve")
                nc.vector.tensor_scalar(ve[:], s2[:], 1.0 / 1024.0, EPS, ALU.mult, ALU.add)
                m2 = lnst_pool.tile([P, 1], f32, tag="m2")
                nc.vector.tensor_mul(out=m2[:], in0=mu[:], in1=mu[:])
                nc.vector.tensor_tensor(ve[:], ve[:], m2[:], ALU.subtract)
                sd = lnst_pool.tile([P, 1], f32, tag="sd")
                nc.scalar.sqrt(sd[:], ve[:])
                rstd = lnst_pool.tile([P, 1], f32, tag="rstd")
                nc.vector.reciprocal(rstd[:], sd[:])
                nmur = lnst_pool.tile([P, 1], f32, tag="nmur")
                nc.vector.tensor_scalar(nmur[:], mu[:], rstd[:, 0:1], -1.0,
                                        ALU.mult, ALU.mult)
                z16 = xn16_pool.tile([P, 1024], f16, tag="z16")
                # normalize split across DVE (low half) and ACT (high half)
                nc.vector.tensor_scalar(z16[:, 0:512], xp[:, 0:512],
                                        rstd[:, 0:1], nmur[:, 0:1],
                                        ALU.mult, ALU.add)
                nc.scalar.activation(z16[:, 512:1024], xp[:, 512:1024], AF.Identity,
                                     bias=nmur[:, 0:1], scale=rstd[:, 0:1])
                zdst = zx_dram if which == 0 else zh_dram
                nc.sync.dma_start(zdst[m * P:(m + 1) * P, :], z16[:])

            for b4 in range(NB4):
                for mloc in range(4):
                    ln_panel(x_d, b4 * 4 + mloc, 0)
                nc.sync.dma_start_transpose(
                    actT[b4][:, 0:8, :], zx_dram[b4 * 512:(b4 + 1) * 512, :])
                for mloc in range(4):
                    ln_panel(h_d, b4 * 4 + mloc, 1)
                nc.sync.dma_start_transpose(
                    actT[b4][:, 8:16, :], zh_dram[b4 * 512:(b4 + 1) * 512, :])

            # ---- Phase 2: gates (oc-major, weights streamed once) ----
            for oc in range(NOC):
                wg = wg_pool.tile([P, KT_G, 512], f16, tag="wg")
                nc.gpsimd.dma_start(wg[:], wg_d[oc].rearrange("ko p c -> p ko c"))
                for b4 in range(NB4):
                    ps = ps_pool.tile([P, 2048], f32, tag="ps")
                    for g in range(4):
                        sl = ps[:, g * 512:(g + 1) * 512]
                        for k in range(KT_G):
                            rhs = actT[b4][:, k, :] if k < 16 else ones_rhs[:]
                            nc.tensor.matmul(sl, wg[:, k, g * P:(g + 1) * P], rhs,
                                             start=(k == 0), stop=(k == KT_G - 1))
                    fio = gate_pool.tile([P, 1536], f32, tag="fio")
                    nc.scalar.activation(fio[:], ps[:, 0:1536], AF.Sigmoid)
                    cp_sb = gate_pool.tile([P, 512], f32, tag="cp")
                    nc.scalar.activation(cp_sb[:], ps[:, 1536:2048], AF.Tanh)

                    cpv = cpv_pool.tile([P, 512], f32, tag="cpv")
                    nc.gpsimd.dma_start(cpv[:], cT_d[oc * P:(oc + 1) * P,
                                                     b4 * 512:(b4 + 1) * 512])
                    f_sb = fio[:, 0:512]
                    i_sb = fio[:, 512:1024]
                    o_sb = fio[:, 1024:1536]
                    nc.vector.tensor_mul(out=f_sb, in0=f_sb, in1=cpv[:])
                    nc.vector.tensor_mul(out=i_sb, in0=i_sb, in1=cp_sb[:])
                    ct = outs_pool.tile([P, 512], f32, tag="ct")
                    nc.vector.tensor_add(out=ct[:], in0=f_sb, in1=i_sb)
                    nc.sync.dma_start(ct_o[oc * P:(oc + 1) * P,
                                           b4 * 512:(b4 + 1) * 512], ct[:])
                    nc.scalar.activation(cp_sb[:], ct[:], AF.Tanh)
                    nc.vector.tensor_mul(out=o_sb, in0=o_sb, in1=cp_sb[:])
                    nc.sync.dma_start(ht_o[oc * P:(oc + 1) * P,
                                           b4 * 512:(b4 + 1) * 512], o_sb)
                    nc.vector.tensor_copy(h16[b4][:, oc, :], o_sb)

            # ---- Phase 3: y (4 n-chunks share one psum; erf on psum) ----
            for b4 in range(NB4):
                for half in range(2):
                    ps = ps_pool.tile([P, 2048], f32, tag="ps")
                    for nn in range(4):
                        n = half * 4 + nn
                        sl = ps[:, nn * 512:(nn + 1) * 512]
                        for k in range(KT_Y):
                            rhs = h16[b4][:, k, :] if k < 8 else ones_rhs[:]
                            nc.tensor.matmul(sl, why[:, k, n * P:(n + 1) * P], rhs,
                                             start=(k == 0), stop=(k == KT_Y - 1))
                    e = gate_pool.tile([P, 2048], f32, tag="fio", name="e")
                    nc.scalar.activation(e[:], ps[:], AF.Erf, scale=SQ2I)
                    nc.vector.tensor_scalar(e[:], e[:], 0.5, 0.5, ALU.mult, ALU.add)
                    nc.vector.tensor_mul(out=e[:], in0=ps[:], in1=e[:])
                    for nn in range(4):
                        n = half * 4 + nn
                        nc.sync.dma_start(yt_o[n * P:(n + 1) * P,
                                               b4 * 512:(b4 + 1) * 512],
                                          e[:, nn * 512:(nn + 1) * 512])

    nc.compile()
    return nc


def _host_prep(inputs):
    f32 = np.float32
    ln_w = inputs["ln_w"].astype(f32)
    ln_b = inputs["ln_b"].astype(f32)
    lnh_w = inputs["lnh_w"].astype(f32)
    lnh_b = inputs["lnh_b"].astype(f32)

    Wx = np.concatenate([inputs["Wxf"], inputs["Wxi"], inputs["Wxo"], inputs["Wxc"]],
                        axis=0).astype(f32)          # [4096, IN]
    Wh = np.concatenate([inputs["Whf"], inputs["Whi"], inputs["Who"], inputs["Whc"]],
                        axis=0).astype(f32)          # [4096, H]
    bg = np.concatenate([inputs["bf"], inputs["bi"], inputs["bo"], inputs["bc"]]).astype(f32)

    # Fold LN affine into weights / bias:
    #   xn @ Wx.T = z_x @ (ln_w * Wx).T + ln_b @ Wx.T
    Wx_eff = Wx * ln_w[None, :]
    Wh_eff = Wh * lnh_w[None, :]
    bias_eff = bg + Wx @ ln_b + Wh @ lnh_b           # [4096]

    Wcat = np.concatenate([Wx_eff.T, Wh_eff.T], axis=0)  # [2048 in', 4096 out]
    # [oc, ko(16), p, (g,j)] with 512-col groups [f_oc|i_oc|o_oc|c_oc]
    Wg16 = (Wcat.reshape(16, P, 4, NOC, P)
                .transpose(3, 0, 1, 2, 4)
                .reshape(NOC, 16, P, 512))
    # bias k-tile: row0 = bias, rest 0
    bias_blk = np.zeros((NOC, 1, P, 512), f32)
    bias_blk[:, 0, 0, :] = (bias_eff.reshape(4, NOC, P)
                            .transpose(1, 0, 2).reshape(NOC, 512))
    Wg = np.concatenate([Wg16, bias_blk], axis=1).astype(np.float16)  # [8,17,128,512]

    WhyT = np.ascontiguousarray(inputs["Why"].astype(f32).T)   # [H, OUT]
    Why16 = WhyT.reshape(8, P, OUT)
    by_blk = np.zeros((1, P, OUT), f32)
    by_blk[0, 0, :] = inputs["by"].astype(f32)
    Why_dev = np.concatenate([Why16, by_blk], axis=0).astype(np.float16)  # [9,128,1024]
    return Wg, Why_dev


def kernel(**inputs):
    global _last_results
    from concourse.bass_utils import run_bass_kernel_spmd

    if "nc" not in _CACHE:
        _CACHE["nc"] = _build_nc()
    nc = _CACHE["nc"]

    Wg, Why_dev = _host_prep(inputs)

    x = np.ascontiguousarray(inputs["x"], dtype=np.float32).reshape(NCORES, BC, IN)
    hp = np.ascontiguousarray(inputs["h_prev"], dtype=np.float32).reshape(NCORES, BC, H)
    cpT = np.ascontiguousarray(
        np.asarray(inputs["c_prev"], dtype=np.float32).reshape(NCORES, BC, H)
        .transpose(0, 2, 1))

    in_maps = [{
        "x": x[c],
        "h_prev": hp[c],
        "c_prevT": cpT[c],
        "Wg": Wg,
        "WhyT": Why_dev,
    } for c in range(NCORES)]

    trace = os.environ.get("LSTM_TRACE", "0") == "1"
    res = run_bass_kernel_spmd(nc, in_maps, core_ids=list(range(NCORES)), trace=trace)
    _last_results = res

    y_t = np.empty((B, OUT), np.float32)
    c_t = np.empty((B, H), np.float32)
    h_t = np.empty((B, H), np.float32)
    for c in range(NCORES):
        r = res.results[c]
        y_t[c * BC:(c + 1) * BC] = r["y_tT"].T
        c_t[c * BC:(c + 1) * BC] = r["c_tT"].T
        h_t[c * BC:(c + 1) * BC] = r["h_tT"].T
    return (y_t, c_t, h_t)
